# revision 1
# baseline (speedup 1.0000x reference)
"""AdaConv kernel for 8 TRN2 NeuronCores — data-parallel over batch.

Math (verified against the reference, rel err 3e-6):
  The reference's per-sample grouped convs collapse:
    - depthwise conv output is identical across the 8 output channels of each
      group: D[n,g,h,w] = sum_{j,kh,kw} d[n,j,kh,kw] * xpad[n,8g+j,h+kh,w+kw]
    - pointwise 1x1 grouped conv collapses to a per-sample scalar
      S[n] = sum_j (s_d @ pk_w.T + pk_b)[n,j]
    - out = leaky(D[n,c//8]*S[n] + bias[n,c]) * (x - mean)/std  (instance norm)

  On device, per core (2 samples, 8 sample-channel-tiles of 128 channels):
    - stencil: 9 accumulating PE matmuls per pixel window with block-diagonal
      weights W_t[k,m] = d[k%8,t] * (k//8 == m//8); output [128, px] in PSUM is
      already replicated across the 8 channels of each group.
    - ScalarE evicts PSUM with fused leaky(S*D + bias) -> predicted (bf16)
    - instance-norm stats: bf16 full-row accumulates with pad-ring correction
      (all Sqrts batched in one op to avoid ACT table thrashing)
    - GPSIMD does the f32->bf16 cast; DVE does stats accums + final multiply
"""

import numpy as np
import ml_dtypes
from contextlib import ExitStack

import concourse.bass as bass
import concourse.tile as tile
from concourse import bacc, mybir
from concourse.bass_utils import run_bass_kernel_spmd

F32 = mybir.dt.float32
BF16 = mybir.dt.bfloat16
AF = mybir.ActivationFunctionType
ALU = mybir.AluOpType
AX = mybir.AxisListType

N_CORES = 8
NSAMP = 2           # samples per core
CH = 512
H = W = 64
PW = 66             # padded row width/height
PPX = PW * PW       # 4356
PPX_AL = PPX + 4    # padded to 4360 so the last stencil window can over-read
NROWV = 7           # output rows per matmul window (7*66=462 <= 512)
WIN = NROWV * PW    # 462
NWIN_FULL = 9       # 9 windows of 7 rows = rows 0..62; window 9 = row 63
TAP_OFF = [kh * PW + kw for kh in range(3) for kw in range(3)]

LAST_RESULTS = None  # BassKernelResults of the most recent run (for test.py)
_CACHE = {}


def _build():
    nc = bacc.Bacc("TRN2", target_bir_lowering=False, debug=False)

    xpad_d = nc.dram_tensor("xpad", [8, 128, PPX_AL], F32, kind="ExternalInput")
    style_d = nc.dram_tensor("style", [128, NSAMP, 4, 16], F32, kind="ExternalInput")
    dwT_d = nc.dram_tensor("dwT", [128, 2, 2, 4, 8], F32, kind="ExternalInput")
    dwb_d = nc.dram_tensor("dwb", [8, 1], F32, kind="ExternalInput")
    pbT_d = nc.dram_tensor("pbT", [128, 4, 512], F32, kind="ExternalInput")
    pbb_d = nc.dram_tensor("pbb", [128, 4], F32, kind="ExternalInput")
    pkwT_d = nc.dram_tensor("pkwT", [128, 4, 8], F32, kind="ExternalInput")
    pkb_d = nc.dram_tensor("pkb", [1, 8], F32, kind="ExternalInput")
    mask_d = nc.dram_tensor("mask", [128, 128], BF16, kind="ExternalInput")
    repl8_d = nc.dram_tensor("repl8", [8, 128], F32, kind="ExternalInput")
    out_d = nc.dram_tensor("out", [8, 128, H * W], BF16, kind="ExternalOutput")

    with tile.TileContext(nc) as tc, ExitStack() as ctx:
        const = ctx.enter_context(tc.tile_pool(name="const", bufs=1))
        small = ctx.enter_context(tc.tile_pool(name="small", bufs=1))
        x32p = ctx.enter_context(tc.tile_pool(name="x32", bufs=3))
        x16p = ctx.enter_context(tc.tile_pool(name="x16", bufs=8))
        junkp = ctx.enter_context(tc.tile_pool(name="junk", bufs=1))
        predp = ctx.enter_context(tc.tile_pool(name="pred", bufs=2))
        xnp = ctx.enter_context(tc.tile_pool(name="xn", bufs=2))
        outp = ctx.enter_context(tc.tile_pool(name="outp", bufs=2))
        psum_main = ctx.enter_context(
            tc.tile_pool(name="psum_main", bufs=2, space="PSUM"))
        psum_small = ctx.enter_context(
            tc.tile_pool(name="psum_small", bufs=2, space="PSUM"))

        # ---- first content loads go out before the params ----
        x32s_pre = []
        for ts in range(2):
            x32 = x32p.tile([128, PPX_AL], F32, tag="x32")
            x32s_pre.append(x32)
            for c in range(4):
                lo, hi = c * 1090, (c + 1) * 1090
                nc.gpsimd.dma_start(x32[:, lo:hi], xpad_d[ts][:, lo:hi])

        # ---- constants / params (single coalesced DMA each) ----
        style_sb = const.tile([128, NSAMP, 4, 16], F32)
        nc.sync.dma_start(style_sb[:], style_d[:])
        dwT_sb = const.tile([128, 2, 2, 4, 8], F32)
        nc.sync.dma_start(dwT_sb[:], dwT_d[:])
        dwb_sb = const.tile([8, 1], F32)
        nc.sync.dma_start(dwb_sb[:], dwb_d[:])
        repl8_sb = const.tile([8, 128], F32)
        nc.sync.dma_start(repl8_sb[:], repl8_d[:])
        mask_sb = const.tile([128, 128], BF16)
        nc.sync.dma_start(mask_sb[:], mask_d[:])
        pkb_sb = const.tile([1, 8], F32)
        nc.scalar.dma_start(pkb_sb[:], pkb_d[:])
        pbb_sb = const.tile([128, 4], F32)
        nc.scalar.dma_start(pbb_sb[:], pbb_d[:])
        pbT_sb = const.tile([128, 4, 512], F32)
        nc.scalar.dma_start(pbT_sb[:], pbT_d[:])
        pkwT_sb = const.tile([128, 4, 8], F32)
        nc.scalar.dma_start(pkwT_sb[:], pkwT_d[:])

        # ---- prologue: kernel-predictor math (all tiny, f32) ----
        W_sb = const.tile([128, NSAMP, 9, 128], BF16)     # stencil weights
        bias_sb = const.tile([128, 4, NSAMP], F32)        # per-channel bias [mt, s]
        Sb_sb = const.tile([128, NSAMP], F32)             # S[n] bcast to 128 parts
        d_sb = small.tile([8, NSAMP, 9], F32)
        dcol_sb = small.tile([128, NSAMP, 9], F32)
        ssum_sb = small.tile([128, 4, NSAMP], F32)        # style sums [kt, s]
        pkwsum_sb = small.tile([128, 4], F32)
        pkbsum_sb = small.tile([1, 1], F32)
        S_sb = small.tile([1, NSAMP], F32)

        eps_sb = const.tile([128, 1], F32)
        nc.vector.memset(eps_sb[:], 1e-5)

        nc.vector.tensor_reduce(pkbsum_sb[:], pkb_sb[:], axis=AX.X, op=ALU.add)
        for kt in range(4):
            nc.vector.tensor_reduce(
                pkwsum_sb[:, kt:kt + 1], pkwT_sb[:, kt, :], axis=AX.X, op=ALU.add)

        for s in range(NSAMP):
            # d = leaky(conv2x2(style, dw_w) + dw_b):  16 accumulating matmuls
            ps_d = psum_small.tile([8, 9], F32, tag="ps_sm")
            i = 0
            for ky in range(2):
                for kx in range(2):
                    for kt in range(4):
                        rhs = style_sb[:, s, kt, :].rearrange(
                            "p (y x) -> p y x", x=4)[:, ky:ky + 3, kx:kx + 3]
                        nc.tensor.matmul(
                            ps_d[:], dwT_sb[:, ky, kx, kt, :], rhs,
                            start=(i == 0), stop=(i == 15))
                        i += 1
            nc.scalar.activation(
                d_sb[:, s, :], ps_d[:], AF.Lrelu, bias=dwb_sb[:], alpha=0.01)

            # replicate d over the 128-channel pattern: dcol[c,t] = d[c%8,t]
            ps_dc = psum_small.tile([128, 9], F32, tag="ps_sm")
            nc.tensor.matmul(ps_dc[:], repl8_sb[:], d_sb[:, s, :])
            nc.vector.tensor_copy(dcol_sb[:, s, :], ps_dc[:])

            # stencil weights W_t = mask * dcol[:, t]
            for t in range(9):
                nc.vector.tensor_scalar(
                    W_sb[:, s, t, :], mask_sb[:], dcol_sb[:, s, t:t + 1], None,
                    ALU.mult)

            # style spatial sums (s_d * 16)
            for kt in range(4):
                nc.vector.tensor_reduce(
                    ssum_sb[:, kt, s:s + 1], style_sb[:, s, kt, :],
                    axis=AX.X, op=ALU.add)

        # bias[c] = s_d @ pb_w[c] + pb_b[c]   (both samples batched, N=2)
        for mt in range(4):
            ps_b = psum_small.tile([128, NSAMP], F32, tag="ps_sm")
            for kt in range(4):
                nc.tensor.matmul(
                    ps_b[:], pbT_sb[:, kt, mt * 128:(mt + 1) * 128],
                    ssum_sb[:, kt, :], start=(kt == 0), stop=(kt == 3))
            nc.scalar.activation(
                bias_sb[:, mt, :], ps_b[:], AF.Identity,
                bias=pbb_sb[:, mt:mt + 1], scale=1.0 / 16.0)

        # S = s_d @ pkw_sum + sum(pk_b)   (both samples, N=2)
        ps_S = psum_small.tile([1, NSAMP], F32, tag="ps_sm")
        for kt in range(4):
            nc.tensor.matmul(
                ps_S[:], pkwsum_sb[:, kt:kt + 1], ssum_sb[:, kt, :],
                start=(kt == 0), stop=(kt == 3))
        nc.scalar.activation(
            S_sb[:], ps_S[:], AF.Identity, bias=pkbsum_sb[:], scale=1.0 / 16.0)
        nc.gpsimd.partition_broadcast(Sb_sb[:], S_sb[:])

        # ---- phase A: load + cast + instance-norm statistics ----
        s_all = small.tile([128, 8], F32)     # per-ts sum(x) over center
        q_all = small.tile([128, 8], F32)     # per-ts sum(x^2) over center
        junkA = junkp.tile([128, H * W], BF16, tag="junkA")
        x16s = []
        for ts in range(8):
            if ts < 2:
                x32 = x32s_pre[ts]
            else:
                x32 = x32p.tile([128, PPX_AL], F32, tag="x32")
                for c in range(4):  # split the 2.2MB load across DMA queues
                    lo, hi = c * 1090, (c + 1) * 1090
                    nc.gpsimd.dma_start(x32[:, lo:hi], xpad_d[ts][:, lo:hi])
            x16 = x16p.tile([128, PPX_AL], BF16, tag="x16")
            x16s.append(x16)
            nc.vector.tensor_copy(x16[:], x32[:])

            x32c = x32[:, :PPX].rearrange("p (h w) -> p h w", w=PW)[:, 1:65, 1:65]
            nc.vector.tensor_reduce(
                s_all[:, ts:ts + 1], x32c, axis=AX.XY, op=ALU.add)
            nc.scalar.activation(
                junkA[:].rearrange("p (h w) -> p h w", w=64), x32c, AF.Square,
                accum_out=q_all[:, ts:ts + 1])

        # batched stats finalize: all [128, 8], one Sqrt (one ACT table set)
        t0_all = small.tile([128, 8], F32)
        nc.vector.tensor_tensor(t0_all[:], s_all[:], s_all[:], ALU.mult)
        u_all = small.tile([128, 8], F32)
        nc.vector.tensor_scalar(
            u_all[:], t0_all[:], -1.0 / 4096.0, None, ALU.mult)
        nc.vector.tensor_tensor(u_all[:], u_all[:], q_all[:], ALU.add)
        stdv_all = small.tile([128, 8], F32)
        nc.scalar.activation(
            stdv_all[:], u_all[:], AF.Sqrt, scale=1.0 / 4095.0, bias=eps_sb[:])
        rstd_all = small.tile([128, 8], F32)
        nc.vector.reciprocal(rstd_all[:], stdv_all[:])
        nmr_all = small.tile([128, 8], F32)
        nc.vector.tensor_tensor(nmr_all[:], rstd_all[:], s_all[:], ALU.mult)
        nc.vector.tensor_scalar(
            nmr_all[:], nmr_all[:], -1.0 / 4096.0, None, ALU.mult)

        # ---- phase B: stencil + predicted + normalize + multiply ----
        for ts in range(8):
            s, q = ts // 4, ts % 4
            x16 = x16s[ts]

            # normalized content (bf16): xn = x*rstd + nmr   (DVE, 1x strided)
            xn = xnp.tile([128, H * W], BF16, tag="xn")
            x16c = x16[:, :PPX].rearrange("p (h w) -> p h w", w=PW)[:, 1:65, 1:65]
            nc.vector.tensor_scalar(
                xn[:].rearrange("p (h w) -> p h w", w=64), x16c,
                rstd_all[:, ts:ts + 1], nmr_all[:, ts:ts + 1],
                ALU.mult, ALU.add)

            pred = predp.tile([128, H * W], BF16, tag="pred")
            # 10 stencil windows; 3 windows per 3-bank PSUM tile
            for f in range(4):
                wins = range(3 * f, min(3 * f + 3, 10))
                psD = psum_main.tile([128, 1536], F32, tag="psD")
                for slot, w in enumerate(wins):
                    n = WIN if w < NWIN_FULL else PW
                    base = NROWV * PW * w
                    for t in range(9):
                        nc.tensor.matmul(
                            psD[:, 512 * slot: 512 * slot + n],
                            W_sb[:, s, t, :],
                            x16[:, base + TAP_OFF[t]: base + TAP_OFF[t] + n],
                            start=(t == 0), stop=(t == 8))
                # fused leaky(S*D + bias), strided to skip the 2-col seams
                if f < 3:
                    src = psD[:].rearrange("p (k x) -> p k x", x=512)[:, :, :WIN] \
                        .rearrange("p k (r w) -> p k r w", w=PW)[:, :, :, :64]
                    dst = pred[:, 1344 * f: 1344 * (f + 1)]
                else:
                    src = psD[:, :64]
                    dst = pred[:, 4032:4096]
                nc.scalar.activation(
                    dst, src, AF.Lrelu,
                    bias=bias_sb[:, q, s:s + 1], scale=Sb_sb[:, s:s + 1],
                    alpha=0.01)

            out_sb = outp.tile([128, H * W], BF16, tag="out")
            nc.vector.tensor_tensor(out_sb[:], pred[:], xn[:], ALU.mult)
            for c in range(4):
                lo, hi = c * 1024, (c + 1) * 1024
                nc.sync.dma_start(out_d[ts][:, lo:hi], out_sb[:, lo:hi])

    nc.compile()
    return nc


def _host_prep(style_encoding, content_in, dw_w, dw_b, pk_w, pk_b, pb_w, pb_b):
    """Shard + lay out inputs for the 8 cores (layout only, no math)."""
    f32 = np.float32
    common = {
        # dwT[c_lo, ky, kx, kt, p] = dw_w[p, kt*128+c_lo, ky, kx]
        "dwT": np.ascontiguousarray(
            dw_w.reshape(8, 4, 128, 2, 2).transpose(2, 3, 4, 1, 0), f32),
        "dwb": np.ascontiguousarray(dw_b.reshape(8, 1), f32),
        # pbT[c_lo, kt, m] = pb_w[m, kt*128+c_lo]
        "pbT": np.ascontiguousarray(
            pb_w.T.reshape(4, 128, 512).transpose(1, 0, 2), f32),
        "pbb": np.ascontiguousarray(pb_b.reshape(4, 128).T, f32),
        "pkwT": np.ascontiguousarray(
            pk_w.T.reshape(4, 128, 8).transpose(1, 0, 2), f32),
        "pkb": np.ascontiguousarray(pk_b.reshape(1, 8), f32),
    }
    ii = np.arange(128)
    common["mask"] = ((ii[:, None] // 8) == (ii[None, :] // 8)).astype(
        ml_dtypes.bfloat16)
    common["repl8"] = (np.arange(8)[:, None] == (ii[None, :] % 8)).astype(f32)

    in_maps = []
    for i in range(N_CORES):
        x = content_in[NSAMP * i: NSAMP * (i + 1)]
        xp = np.pad(x, ((0, 0), (0, 0), (1, 1), (1, 1)), mode="reflect")
        xp = xp.reshape(NSAMP, 4, 128, PPX)
        xp = np.concatenate(
            [xp, np.zeros((NSAMP, 4, 128, PPX_AL - PPX), f32)], axis=-1)
        se = style_encoding[NSAMP * i: NSAMP * (i + 1)]
        in_maps.append({
            "xpad": np.ascontiguousarray(xp.reshape(8, 128, PPX_AL), f32),
            # style[c_lo, s, kt, yx] = se[s, kt*128+c_lo, yx]
            "style": np.ascontiguousarray(
                se.reshape(NSAMP, 4, 128, 16).transpose(2, 0, 1, 3), f32),
            **common,
        })
    return in_maps


def kernel(style_encoding, content_in, dw_w, dw_b, pk_w, pk_b, pb_w, pb_b):
    global LAST_RESULTS
    import os
    if "nc" not in _CACHE:
        _CACHE["nc"] = _build()
    nc = _CACHE["nc"]
    in_maps = _host_prep(style_encoding, content_in, dw_w, dw_b,
                         pk_w, pk_b, pb_w, pb_b)
    res = run_bass_kernel_spmd(
        nc, in_maps, core_ids=list(range(N_CORES)),
        trace=bool(os.environ.get("ADACONV_TRACE")))
    LAST_RESULTS = res
    outs = []
    for i in range(N_CORES):
        o = np.asarray(res.results[i]["out"]).astype(np.float32)
        outs.append(o.reshape(NSAMP, 4, 128, 64, 64).reshape(NSAMP, CH, 64, 64))
    return np.concatenate(outs, axis=0)



# revision 7
# speedup vs baseline: 1.1557x; 1.1557x over previous
"""AdaConv kernel for 8 TRN2 NeuronCores — data-parallel over batch.

Two-stage stencil formulation (v2). Math identical to the reference after
collapsing the per-sample grouped convs:
    D[n,g,h,w] = sum_{j,kh,kw} d[n,j,kh,kw] * xpad[n,8g+j,h+kh,w+kw]
    out = leaky(S[n]*D[n,c//8] + bias[n,c]) * (x - mean)/std

Per core (2 samples, 8 tiles of 128 channels; x stored bf16, pitch-68 rows):
  stage 1 (PE): 3 accumulating matmuls (kw taps via rhs column offsets),
      M = 96 = (kh,g) pairs 32-aligned -> P2[(kh,g), p] in PSUM.
      Columns consumed: 3*4488 per tile (vs 9*4224 for the 1-stage form).
  fold (ACT+DMA): PSUM->SBUF copy (bf16), then 2 SBUF->SBUF DMAs shift the
      kh=1,2 slabs by kh*68 columns so the tap alignment is pre-applied.
  stage 2 (PE): ONE K=48 matmul per row-chunk -> D replicated over the 8
      channels of each group, directly in [128, px] layout.
  ScalarE evicts with fused leaky(S*rstd*D + bias*rstd) (rstd folded in:
      leaky(z)*r == leaky(z*r) for r>0).
  Stats: bn_stats/bn_aggr on DVE (one pass -> mean+var from bf16 x).
  Final: one DVE scalar_tensor_tensor: out = (x - mean) * predn.
"""

import numpy as np
import ml_dtypes
from contextlib import ExitStack

import concourse.bass as bass
import concourse.tile as tile
from concourse import bacc, mybir
from concourse.bass_utils import run_bass_kernel_spmd

F32 = mybir.dt.float32
BF16 = mybir.dt.bfloat16
AF = mybir.ActivationFunctionType
ALU = mybir.AluOpType
AX = mybir.AxisListType

N_CORES = 8
NSAMP = 2           # samples per core
CH = 512
H = W = 64
PW = 68             # row pitch (junk col 0, padded cols 1..66, junk col 67)
NR = 66             # padded rows
EXT = PW * NR       # 4488 flat extent
XAL = EXT + 8       # 4496 allocated (stencil over-read + even)
C1 = 1024           # stage-1 psum chunk (2 banks, rhs bf16 max)
RCH = 7             # stage-2 rows per matmul slot (7*68 = 476 <= 512)

LAST_RESULTS = None  # BassKernelResults of the most recent run (for test.py)
_CACHE = {}


def _build():
    nc = bacc.Bacc("TRN2", target_bir_lowering=False, debug=False)

    x16_d = nc.dram_tensor("x16", [8, 128, XAL], BF16, kind="ExternalInput")
    style_d = nc.dram_tensor("style", [128, NSAMP, 4, 16], F32, kind="ExternalInput")
    dwT_d = nc.dram_tensor("dwT", [128, 2, 2, 4, 8], F32, kind="ExternalInput")
    dwb_d = nc.dram_tensor("dwb", [8, 1], F32, kind="ExternalInput")
    pbT_d = nc.dram_tensor("pbT", [128, 4, 512], F32, kind="ExternalInput")
    pbb_d = nc.dram_tensor("pbb", [128, 4], F32, kind="ExternalInput")
    pkwT_d = nc.dram_tensor("pkwT", [128, 4, 8], F32, kind="ExternalInput")
    pkb_d = nc.dram_tensor("pkb", [1, 8], F32, kind="ExternalInput")
    mask32_d = nc.dram_tensor("mask32", [128, 32], BF16, kind="ExternalInput")
    w2_d = nc.dram_tensor("w2", [48, 128], BF16, kind="ExternalInput")
    repl8_d = nc.dram_tensor("repl8", [8, 128], F32, kind="ExternalInput")
    out_d = nc.dram_tensor("out", [8, 128, H * W], BF16, kind="ExternalOutput")

    with tile.TileContext(nc) as tc, ExitStack() as ctx:
        const = ctx.enter_context(tc.tile_pool(name="const", bufs=1))
        small = ctx.enter_context(tc.tile_pool(name="small", bufs=1))
        x16p = ctx.enter_context(tc.tile_pool(name="x16", bufs=8))
        p2p = ctx.enter_context(tc.tile_pool(name="p2", bufs=2))
        statp = ctx.enter_context(tc.tile_pool(name="stat", bufs=2))
        prednp = ctx.enter_context(tc.tile_pool(name="pred", bufs=2))
        outp = ctx.enter_context(tc.tile_pool(name="outp", bufs=2))
        psumA = ctx.enter_context(
            tc.tile_pool(name="psumA", bufs=2, space="PSUM"))
        psumB = ctx.enter_context(
            tc.tile_pool(name="psumB", bufs=2, space="PSUM"))

        # ---- params (small, first on the sync queue) ----
        style_sb = const.tile([128, NSAMP, 4, 16], F32)
        nc.sync.dma_start(style_sb[:], style_d[:])
        dwT_sb = const.tile([128, 2, 2, 4, 8], F32)
        nc.sync.dma_start(dwT_sb[:], dwT_d[:])
        dwb_sb = const.tile([8, 1], F32)
        nc.sync.dma_start(dwb_sb[:], dwb_d[:])
        repl8_sb = const.tile([8, 128], F32)
        nc.sync.dma_start(repl8_sb[:], repl8_d[:])
        mask32_sb = const.tile([128, 32], BF16)
        nc.sync.dma_start(mask32_sb[:], mask32_d[:])
        w2_sb = const.tile([48, 128], BF16)
        nc.sync.dma_start(w2_sb[:], w2_d[:])
        pkb_sb = const.tile([1, 8], F32)
        nc.scalar.dma_start(pkb_sb[:], pkb_d[:])
        pbb_sb = const.tile([128, 4], F32)
        nc.scalar.dma_start(pbb_sb[:], pbb_d[:])
        pbT_sb = const.tile([128, 4, 512], F32)
        nc.scalar.dma_start(pbT_sb[:], pbT_d[:])
        pkwT_sb = const.tile([128, 4, 8], F32)
        nc.scalar.dma_start(pkwT_sb[:], pkwT_d[:])

        # ---- content loads: 4 split DMAs per tile on the sync queue ----
        x16s = []
        for ts in range(8):
            x16 = x16p.tile([128, XAL], BF16, tag="x16")
            x16s.append(x16)
            for c in range(4):
                lo, hi = c * 1124, (c + 1) * 1124
                nc.sync.dma_start(x16[:, lo:hi], x16_d[ts][:, lo:hi])

        # ---- prologue: kernel-predictor math (tiny, f32) ----
        W1_sb = const.tile([128, NSAMP, 3, 96], BF16)     # stage-1 weights
        bias_sb = const.tile([128, 4, NSAMP], F32)        # per-channel bias
        Sb_sb = const.tile([128, NSAMP], F32)             # S[n] on 128 parts
        d_sb = small.tile([8, NSAMP, 9], F32)
        dcol_sb = small.tile([128, NSAMP, 9], F32)
        ssum_sb = small.tile([128, 4, NSAMP], F32)
        pkwsum_sb = small.tile([128, 4], F32)
        pkbsum_sb = small.tile([1, 1], F32)
        S_sb = small.tile([1, NSAMP], F32)
        eps_sb = const.tile([128, 1], F32)
        nc.vector.memset(eps_sb[:], 1e-5)
        nc.vector.memset(W1_sb[:], 0.0)

        nc.vector.tensor_reduce(pkbsum_sb[:], pkb_sb[:], axis=AX.X, op=ALU.add)
        for kt in range(4):
            nc.vector.tensor_reduce(
                pkwsum_sb[:, kt:kt + 1], pkwT_sb[:, kt, :], axis=AX.X, op=ALU.add)

        for s in range(NSAMP):
            # d = leaky(conv2x2(style, dw_w) + dw_b):  16 accumulating matmuls
            psA0 = psumA.tile([128, C1], F32, tag="psA")
            ps_d = psA0[0:8, 0:9]
            i = 0
            for ky in range(2):
                for kx in range(2):
                    for kt in range(4):
                        rhs = style_sb[:, s, kt, :].rearrange(
                            "p (y x) -> p y x", x=4)[:, ky:ky + 3, kx:kx + 3]
                        nc.tensor.matmul(
                            ps_d, dwT_sb[:, ky, kx, kt, :], rhs,
                            start=(i == 0), stop=(i == 15))
                        i += 1
            nc.scalar.activation(
                d_sb[:, s, :], ps_d, AF.Lrelu, bias=dwb_sb[:], alpha=0.01)

            # replicate d over channels: dcol[c,t] = d[c%8,t]
            psA1 = psumA.tile([128, C1], F32, tag="psA")
            ps_dc = psA1[:, 0:9]
            nc.tensor.matmul(ps_dc, repl8_sb[:], d_sb[:, s, :])
            nc.vector.tensor_copy(dcol_sb[:, s, :], ps_dc)

            # stage-1 weights W1[kw][ch, kh*32+g] = d[ch%8, kh, kw]*(g==ch//8)
            for kh in range(3):
                for kw in range(3):
                    nc.vector.tensor_scalar(
                        W1_sb[:, s, kw, kh * 32: kh * 32 + 32], mask32_sb[:],
                        dcol_sb[:, s, 3 * kh + kw: 3 * kh + kw + 1], None,
                        ALU.mult)

            # style spatial sums (s_d * 16)
            for kt in range(4):
                nc.vector.tensor_reduce(
                    ssum_sb[:, kt, s:s + 1], style_sb[:, s, kt, :],
                    axis=AX.X, op=ALU.add)

        # bias[c] = s_d @ pb_w[c] + pb_b[c]   (both samples batched)
        for mt in range(4):
            psB0 = psumB.tile([128, 1024], F32, tag="psB")
            ps_b = psB0[:, 0:NSAMP]
            for kt in range(4):
                nc.tensor.matmul(
                    ps_b, pbT_sb[:, kt, mt * 128:(mt + 1) * 128],
                    ssum_sb[:, kt, :], start=(kt == 0), stop=(kt == 3))
            nc.scalar.activation(
                bias_sb[:, mt, :], ps_b, AF.Identity,
                bias=pbb_sb[:, mt:mt + 1], scale=1.0 / 16.0)

        # S = s_d @ pkw_sum + sum(pk_b)
        psB1 = psumB.tile([128, 1024], F32, tag="psB")
        ps_S = psB1[0:1, 0:NSAMP]
        for kt in range(4):
            nc.tensor.matmul(
                ps_S, pkwsum_sb[:, kt:kt + 1], ssum_sb[:, kt, :],
                start=(kt == 0), stop=(kt == 3))
        nc.scalar.activation(
            S_sb[:], ps_S, AF.Identity, bias=pkbsum_sb[:], scale=1.0 / 16.0)
        nc.gpsimd.partition_broadcast(Sb_sb[:], S_sb[:])

        # ---- per-tile state ----
        s_all = small.tile([128, 8], F32)            # sum(x) per tile
        q_all = small.tile([128, 8], F32)            # sum(x^2) per tile
        mu_all = small.tile([128, 8], F32)
        u_all = small.tile([128, 8], F32)
        stdv_all = small.tile([128, 8], F32)
        rstd_all = small.tile([128, 8], F32)
        scaleS_all = small.tile([128, 8], F32)       # S * rstd
        biasS_all = small.tile([128, 8], F32)        # bias * rstd
        junk_sb = small.tile([128, H * W], BF16)     # stats junk writes

        def emit_stats(ts):
            s, kt = ts // 4, ts % 4
            x16 = x16s[ts]
            xc = x16[:, :EXT].rearrange(
                "p (r w) -> p r w", w=PW)[:, 1:65, 2:66]
            jv = junk_sb[:].rearrange("p (r w) -> p r w", w=64)
            # sum(x): out = max(x+0, x); accum = sum(out)
            nc.vector.scalar_tensor_tensor(
                jv, xc, 0.0, xc, ALU.add, ALU.max,
                accum_out=s_all[:, ts:ts + 1])
            # sum(x^2): out = (x+0)*x
            nc.vector.scalar_tensor_tensor(
                jv, xc, 0.0, xc, ALU.add, ALU.mult,
                accum_out=q_all[:, ts:ts + 1])
            # var_unb*4095 = q - s^2/4096 ; stdv = sqrt(u/4095 + eps)
            nc.vector.tensor_scalar(
                mu_all[:, ts:ts + 1], s_all[:, ts:ts + 1], 1.0 / 4096.0,
                None, ALU.mult)
            nc.vector.tensor_tensor(
                u_all[:, ts:ts + 1], mu_all[:, ts:ts + 1],
                s_all[:, ts:ts + 1], ALU.mult)
            nc.vector.tensor_tensor(
                u_all[:, ts:ts + 1], q_all[:, ts:ts + 1],
                u_all[:, ts:ts + 1], ALU.subtract)
            nc.scalar.activation(
                stdv_all[:, ts:ts + 1], u_all[:, ts:ts + 1], AF.Sqrt,
                scale=1.0 / 4095.0, bias=eps_sb[:])
            nc.vector.reciprocal(rstd_all[:, ts:ts + 1], stdv_all[:, ts:ts + 1])
            nc.vector.tensor_scalar(
                scaleS_all[:, ts:ts + 1], rstd_all[:, ts:ts + 1],
                Sb_sb[:, s:s + 1], None, ALU.mult)
            nc.vector.tensor_scalar(
                biasS_all[:, ts:ts + 1], rstd_all[:, ts:ts + 1],
                bias_sb[:, kt, s:s + 1], None, ALU.mult)

        def emit_stage1(ts):
            s = ts // 4
            x16 = x16s[ts]
            p2 = p2p.tile([96, XAL], BF16, tag="p2")
            for c in range(0, EXT, C1):
                tw = min(C1, EXT - c)
                psA = psumA.tile([128, C1], F32, tag="psA")
                for sub in (0, 512):
                    lo = c + sub
                    cw = min(512, EXT - lo)
                    if cw <= 0:
                        continue
                    for kw in range(3):
                        nc.tensor.matmul(
                            psA[0:96, sub:sub + cw], W1_sb[:, s, kw, :],
                            x16[:, lo + kw: lo + kw + cw],
                            start=(kw == 0), stop=(kw == 2))
                nc.scalar.copy(p2[:, c:c + tw], psA[0:96, :tw])
            # pre-apply kh row shifts (overlay: kh1 -> rows 16:32, kh2 -> 32:48)
            nc.gpsimd.dma_start(p2[16:32, 0:EXT - PW], p2[32:48, PW:EXT])
            nc.gpsimd.dma_start(
                p2[32:48, 0:EXT - 2 * PW], p2[64:80, 2 * PW:EXT])
            return p2

        def emit_stage2(ts, p2):
            x16 = x16s[ts]
            xr = x16[:, :EXT].rearrange("p (r w) -> p r w", w=PW)
            predn = prednp.tile([128, H * W], BF16, tag="pred")
            out_sb = outp.tile([128, H * W], BF16, tag="out")
            for rt in range(0, 64, 2 * RCH):
                psB = psumB.tile([128, 1024], F32, tag="psB")
                for sub in (0, 512):
                    r0 = rt + (RCH if sub else 0)
                    nr = min(RCH, 64 - r0)
                    if nr <= 0:
                        continue
                    nc.tensor.matmul(
                        psB[:, sub:sub + nr * PW], w2_sb[:],
                        p2[0:48, r0 * PW + 1: (r0 + nr) * PW + 1],
                        start=True, stop=True)
                    src = psB[:, sub:sub + nr * PW].rearrange(
                        "p (r w) -> p r w", w=PW)[:, :, 0:64]
                    dst = predn[:, r0 * 64:(r0 + nr) * 64].rearrange(
                        "p (r w) -> p r w", w=64)
                    nc.scalar.activation(
                        dst, src, AF.Lrelu,
                        bias=biasS_all[:, ts:ts + 1],
                        scale=scaleS_all[:, ts:ts + 1], alpha=0.01)
                    # out = (x - mean) * predn   (one fused DVE op)
                    nc.vector.scalar_tensor_tensor(
                        out_sb[:, r0 * 64:(r0 + nr) * 64].rearrange(
                            "p (r w) -> p r w", w=64),
                        xr[:, 1 + r0: 1 + r0 + nr, 2:66],
                        mu_all[:, ts:ts + 1],
                        dst,
                        ALU.subtract, ALU.mult)
                nrt = min(2 * RCH, 64 - rt)
                nc.gpsimd.dma_start(
                    out_d[ts][:, rt * 64:(rt + nrt) * 64],
                    out_sb[:, rt * 64:(rt + nrt) * 64])

        # ---- software-pipelined main loop: stage1(k+1) ahead of stage2(k) ----
        emit_stats(0)
        p2_prev = emit_stage1(0)
        for k in range(8):
            if k + 1 < 8:
                emit_stats(k + 1)
                p2_next = emit_stage1(k + 1)
            emit_stage2(k, p2_prev)
            if k + 1 < 8:
                p2_prev = p2_next

    nc.compile()
    return nc


def _host_prep(style_encoding, content_in, dw_w, dw_b, pk_w, pk_b, pb_w, pb_b):
    """Shard + lay out inputs for the 8 cores (layout only, no math)."""
    f32 = np.float32
    bf = ml_dtypes.bfloat16
    common = {
        "dwT": np.ascontiguousarray(
            dw_w.reshape(8, 4, 128, 2, 2).transpose(2, 3, 4, 1, 0), f32),
        "dwb": np.ascontiguousarray(dw_b.reshape(8, 1), f32),
        "pbT": np.ascontiguousarray(
            pb_w.T.reshape(4, 128, 512).transpose(1, 0, 2), f32),
        "pbb": np.ascontiguousarray(pb_b.reshape(4, 128).T, f32),
        "pkwT": np.ascontiguousarray(
            pk_w.T.reshape(4, 128, 8).transpose(1, 0, 2), f32),
        "pkb": np.ascontiguousarray(pk_b.reshape(1, 8), f32),
    }
    ii = np.arange(128)
    common["mask32"] = (np.arange(32)[None, :] == (ii[:, None] // 8)).astype(bf)
    w2 = np.zeros((48, 128), bf)
    for kh in range(3):
        w2[kh * 16 + ii // 8, ii] = 1
    common["w2"] = w2
    common["repl8"] = (np.arange(8)[:, None] == (ii[None, :] % 8)).astype(f32)

    # padded pitch-68 bf16 content, all cores at once
    xp = np.pad(content_in, ((0, 0), (0, 0), (1, 1), (1, 1)), mode="reflect")
    buf = np.zeros((16, CH, NR, PW), f32)
    buf[:, :, :, 1:67] = xp
    xb = buf.reshape(16, 4, 128, EXT).astype(bf)

    in_maps = []
    for i in range(N_CORES):
        x16 = np.zeros((NSAMP, 4, 128, XAL), bf)
        x16[:, :, :, :EXT] = xb[NSAMP * i: NSAMP * (i + 1)]
        se = style_encoding[NSAMP * i: NSAMP * (i + 1)]
        in_maps.append({
            "x16": np.ascontiguousarray(x16.reshape(8, 128, XAL)),
            "style": np.ascontiguousarray(
                se.reshape(NSAMP, 4, 128, 16).transpose(2, 0, 1, 3), f32),
            **common,
        })
    return in_maps


def kernel(style_encoding, content_in, dw_w, dw_b, pk_w, pk_b, pb_w, pb_b):
    global LAST_RESULTS
    import os
    if "nc" not in _CACHE:
        _CACHE["nc"] = _build()
    nc = _CACHE["nc"]
    in_maps = _host_prep(style_encoding, content_in, dw_w, dw_b,
                         pk_w, pk_b, pb_w, pb_b)
    res = run_bass_kernel_spmd(
        nc, in_maps, core_ids=list(range(N_CORES)),
        trace=bool(os.environ.get("ADACONV_TRACE")))
    LAST_RESULTS = res
    outs = []
    for i in range(N_CORES):
        o = np.asarray(res.results[i]["out"]).astype(np.float32)
        outs.append(o.reshape(NSAMP, 4, 128, 64, 64).reshape(NSAMP, CH, 64, 64))
    return np.concatenate(outs, axis=0)


# revision 10
# speedup vs baseline: 1.3537x; 1.1713x over previous
"""AdaConv kernel for 8 TRN2 NeuronCores — data-parallel over batch.

Two-stage stencil formulation. Math identical to the reference after
collapsing the per-sample grouped convs:
    D[n,g,h,w] = sum_{j,kh,kw} d[n,j,kh,kw] * xpad[n,8g+j,h+kh,w+kw]
    out = leaky(S[n]*D[n,c//8] + bias[n,c]) * (x - mean)/std

Per core (2 samples, 8 tiles of 128 channels; x stored bf16, pitch-68 rows):
  stage 1 (PE): 3 accumulating matmuls (kw taps via rhs column offsets),
      M = 96 = (kh,g) pairs 32-aligned -> P2[(kh,g), p] in PSUM.
  fold: PSUM->SBUF copy drops the seam cols (dense 64-pitch bf16), then two
      SBUF->SBUF DMAs shift the kh=1,2 slabs by kh*64 so taps align.
  stage 2 (PE): ONE K=48 matmul per 7-row slot -> D replicated over the 8
      channels of each group, directly in [128, px] dense layout.
  ScalarE evicts with fused leaky(S*rstd*D + bias*rstd) (valid: r>0).
  Stats on DVE: bn_stats/bn_aggr (bf16 x); rstd via Newton rsqrt on DVE
      (no ScalarE Sqrt -> no activation-table thrash).
  Final: DVE tensor_scalar xn = x - mean, tensor_tensor out = xn * predn.
"""

import numpy as np
import ml_dtypes
from contextlib import ExitStack

import concourse.bass as bass
import concourse.tile as tile
from concourse import bacc, mybir
from concourse.bass_utils import run_bass_kernel_spmd

F32 = mybir.dt.float32
I32 = mybir.dt.int32
BF16 = mybir.dt.bfloat16
AF = mybir.ActivationFunctionType
ALU = mybir.AluOpType
AX = mybir.AxisListType

N_CORES = 8
NSAMP = 2           # samples per core
CH = 512
H = W = 64
PW = 68             # row pitch (junk col 0, padded cols 1..66, junk col 67)
NR = 66             # padded rows
EXT = PW * NR       # 4488 flat extent
XAL = EXT + 8       # 4496 allocated (stencil over-read + even)
DE = 64 * NR        # 4224 dense P2 extent
RSQRT_MAGIC = np.int32(0x5F3759DF).view(np.float32).item()

LAST_RESULTS = None  # BassKernelResults of the most recent run (for test.py)
_CACHE = {}


def _build():
    nc = bacc.Bacc("TRN2", target_bir_lowering=False, debug=False)

    x16_d = nc.dram_tensor("x16", [8, 128, XAL], BF16, kind="ExternalInput")
    style_d = nc.dram_tensor("style", [128, NSAMP, 4, 16], F32, kind="ExternalInput")
    dwT_d = nc.dram_tensor("dwT", [128, 2, 2, 4, 8], F32, kind="ExternalInput")
    dwb_d = nc.dram_tensor("dwb", [8, 1], F32, kind="ExternalInput")
    pbT_d = nc.dram_tensor("pbT", [128, 4, 512], F32, kind="ExternalInput")
    pbb_d = nc.dram_tensor("pbb", [128, 4], F32, kind="ExternalInput")
    pkwT_d = nc.dram_tensor("pkwT", [128, 4, 8], F32, kind="ExternalInput")
    pkb_d = nc.dram_tensor("pkb", [1, 8], F32, kind="ExternalInput")
    mask32_d = nc.dram_tensor("mask32", [128, 32], BF16, kind="ExternalInput")
    w2_d = nc.dram_tensor("w2", [48, 128], BF16, kind="ExternalInput")
    repl8_d = nc.dram_tensor("repl8", [8, 128], F32, kind="ExternalInput")
    out_d = nc.dram_tensor("out", [8, 128, H * W], BF16, kind="ExternalOutput")

    with tile.TileContext(nc) as tc, ExitStack() as ctx:
        const = ctx.enter_context(tc.tile_pool(name="const", bufs=1))
        small = ctx.enter_context(tc.tile_pool(name="small", bufs=1))
        x16p = ctx.enter_context(tc.tile_pool(name="x16", bufs=8))
        p2p = ctx.enter_context(tc.tile_pool(name="p2", bufs=2))
        statp = ctx.enter_context(tc.tile_pool(name="stat", bufs=2))
        xnp = ctx.enter_context(tc.tile_pool(name="xn", bufs=2))
        prednp = ctx.enter_context(tc.tile_pool(name="pred", bufs=2))
        outp = ctx.enter_context(tc.tile_pool(name="outp", bufs=2))
        psumA = ctx.enter_context(
            tc.tile_pool(name="psumA", bufs=2, space="PSUM"))
        psumB = ctx.enter_context(
            tc.tile_pool(name="psumB", bufs=2, space="PSUM"))

        # ---- params (small, first on the sync queue) ----
        style_sb = const.tile([128, NSAMP, 4, 16], F32)
        nc.sync.dma_start(style_sb[:], style_d[:])
        dwT_sb = const.tile([128, 2, 2, 4, 8], F32)
        nc.sync.dma_start(dwT_sb[:], dwT_d[:])
        dwb_sb = const.tile([8, 1], F32)
        nc.sync.dma_start(dwb_sb[:], dwb_d[:])
        repl8_sb = const.tile([8, 128], F32)
        nc.sync.dma_start(repl8_sb[:], repl8_d[:])
        mask32_sb = const.tile([128, 32], BF16)
        nc.sync.dma_start(mask32_sb[:], mask32_d[:])
        w2_sb = const.tile([48, 128], BF16)
        nc.sync.dma_start(w2_sb[:], w2_d[:])
        pkb_sb = const.tile([1, 8], F32)
        nc.scalar.dma_start(pkb_sb[:], pkb_d[:])
        pbb_sb = const.tile([128, 4], F32)
        nc.scalar.dma_start(pbb_sb[:], pbb_d[:])
        pbT_sb = const.tile([128, 4, 512], F32)
        nc.scalar.dma_start(pbT_sb[:], pbT_d[:])
        pkwT_sb = const.tile([128, 4, 8], F32)
        nc.scalar.dma_start(pkwT_sb[:], pkwT_d[:])

        # content loads for the first tiles (prefetch window of 3)
        x16s = []
        for _ in range(8):
            x16 = x16p.tile([128, XAL], BF16, tag="x16")
            x16s.append(x16)

        def emit_xin(ts):
            for c in range(4):
                lo, hi = c * 1124, (c + 1) * 1124
                nc.sync.dma_start(x16s[ts][:, lo:hi], x16_d[ts][:, lo:hi])

        for ts in range(3):
            emit_xin(ts)

        # ---- prologue: kernel-predictor math (tiny, f32) ----
        W1_sb = const.tile([128, NSAMP, 3, 96], BF16)     # stage-1 weights
        bias_sb = const.tile([128, 4, NSAMP], F32)        # per-channel bias
        Sb_sb = const.tile([128, NSAMP], F32)             # S[n] on 128 parts
        d_sb = small.tile([8, NSAMP, 9], F32)
        dcol_sb = small.tile([128, NSAMP, 9], F32)
        ssum_sb = small.tile([128, 4, NSAMP], F32)
        pkwsum_sb = small.tile([128, 4], F32)
        pkbsum_sb = small.tile([1, 1], F32)
        S_sb = small.tile([1, NSAMP], F32)
        magic_sb = const.tile([128, 1], F32)
        nc.vector.memset(magic_sb[:], RSQRT_MAGIC)
        nc.vector.memset(W1_sb[:], 0.0)

        nc.vector.tensor_reduce(pkbsum_sb[:], pkb_sb[:], axis=AX.X, op=ALU.add)
        for kt in range(4):
            nc.vector.tensor_reduce(
                pkwsum_sb[:, kt:kt + 1], pkwT_sb[:, kt, :], axis=AX.X, op=ALU.add)

        for s in range(NSAMP):
            # d = leaky(conv2x2(style, dw_w) + dw_b):  16 accumulating matmuls
            psA0 = psumA.tile([128, 1024], F32, tag="psA")
            ps_d = psA0[0:8, 0:9]
            i = 0
            for ky in range(2):
                for kx in range(2):
                    for kt in range(4):
                        rhs = style_sb[:, s, kt, :].rearrange(
                            "p (y x) -> p y x", x=4)[:, ky:ky + 3, kx:kx + 3]
                        nc.tensor.matmul(
                            ps_d, dwT_sb[:, ky, kx, kt, :], rhs,
                            start=(i == 0), stop=(i == 15))
                        i += 1
            nc.scalar.activation(
                d_sb[:, s, :], ps_d, AF.Lrelu, bias=dwb_sb[:], alpha=0.01)

            # replicate d over channels: dcol[c,t] = d[c%8,t]
            psA1 = psumA.tile([128, 1024], F32, tag="psA")
            ps_dc = psA1[:, 0:9]
            nc.tensor.matmul(ps_dc, repl8_sb[:], d_sb[:, s, :])
            nc.vector.tensor_copy(dcol_sb[:, s, :], ps_dc)

            # stage-1 weights W1[kw][ch, kh*32+g] = d[ch%8, kh, kw]*(g==ch//8)
            for kh in range(3):
                for kw in range(3):
                    nc.vector.tensor_scalar(
                        W1_sb[:, s, kw, kh * 32: kh * 32 + 32], mask32_sb[:],
                        dcol_sb[:, s, 3 * kh + kw: 3 * kh + kw + 1], None,
                        ALU.mult)

            # style spatial sums (s_d * 16)
            for kt in range(4):
                nc.vector.tensor_reduce(
                    ssum_sb[:, kt, s:s + 1], style_sb[:, s, kt, :],
                    axis=AX.X, op=ALU.add)

        # bias[c] = s_d @ pb_w[c] + pb_b[c]   (both samples batched)
        for mt in range(4):
            psB0 = psumB.tile([128, 1024], F32, tag="psB")
            ps_b = psB0[:, 0:NSAMP]
            for kt in range(4):
                nc.tensor.matmul(
                    ps_b, pbT_sb[:, kt, mt * 128:(mt + 1) * 128],
                    ssum_sb[:, kt, :], start=(kt == 0), stop=(kt == 3))
            nc.vector.tensor_scalar(
                bias_sb[:, mt, :], ps_b, 1.0 / 16.0,
                pbb_sb[:, mt:mt + 1], ALU.mult, ALU.add)

        # S = s_d @ pkw_sum + sum(pk_b)
        psB1 = psumB.tile([128, 1024], F32, tag="psB")
        ps_S = psB1[0:1, 0:NSAMP]
        for kt in range(4):
            nc.tensor.matmul(
                ps_S, pkwsum_sb[:, kt:kt + 1], ssum_sb[:, kt, :],
                start=(kt == 0), stop=(kt == 3))
        nc.vector.tensor_scalar(
            S_sb[:], ps_S, 1.0 / 16.0, pkbsum_sb[:], ALU.mult, ALU.add)
        nc.gpsimd.partition_broadcast(Sb_sb[:], S_sb[:])

        # ---- per-tile state ----
        mu_all = small.tile([128, 8], F32)
        v_all = small.tile([128, 8], F32)            # var + eps
        y_all = small.tile([128, 8], F32)            # rsqrt iterate
        t_all = small.tile([128, 8], F32)
        scaleS_all = small.tile([128, 8], F32)       # S * rstd
        biasS_all = small.tile([128, 8], F32)        # bias * rstd

        def bn_stats_raw(out, in_):
            # bass's bn_stats wrapper mis-asserts the out shape for 3D
            # inputs; the HW op always writes 6 elements/partition.
            eng = nc.vector
            return eng.add_instruction(mybir.InstBNStats(
                name=eng.bass.get_next_instruction_name(),
                ins=[eng.lower_ap(in_)], outs=[eng.lower_ap(out)]))

        def emit_stats(ts):
            s, kt = ts // 4, ts % 4
            xr = x16s[ts][:, :EXT].rearrange("p (r w) -> p r w", w=PW)
            st = statp.tile([128, 8, 6], F32, tag="bn")
            for i in range(8):
                bn_stats_raw(st[:, i, :], xr[:, 1 + 8 * i: 9 + 8 * i, 2:66])
            mv = statp.tile([128, 2], F32, tag="mv")
            nc.vector.bn_aggr(mv[:], st[:])
            nc.vector.tensor_copy(mu_all[:, ts:ts + 1], mv[:, 0:1])
            # v = var*4096/4095 + eps
            nc.vector.tensor_scalar(
                v_all[:, ts:ts + 1], mv[:, 1:2], 4096.0 / 4095.0, 1e-5,
                ALU.mult, ALU.add)
            # Newton rsqrt: y0 from the bit trick, then 2 iterations
            vi = v_all[:, ts:ts + 1].bitcast(I32)
            ti = t_all[:, ts:ts + 1].bitcast(I32)
            yi = y_all[:, ts:ts + 1].bitcast(I32)
            nc.vector.tensor_scalar(ti, vi, 1, None, ALU.arith_shift_right)
            nc.vector.tensor_tensor(
                yi, magic_sb[:].bitcast(I32), ti, ALU.subtract)
            y = y_all[:, ts:ts + 1]
            t = t_all[:, ts:ts + 1]
            v = v_all[:, ts:ts + 1]
            for _ in range(2):
                nc.vector.tensor_tensor(t, y, y, ALU.mult)
                nc.vector.tensor_tensor(t, t, v, ALU.mult)
                nc.vector.tensor_scalar(t, t, -0.5, 1.5, ALU.mult, ALU.add)
                nc.vector.tensor_tensor(y, y, t, ALU.mult)
            nc.vector.tensor_scalar(
                scaleS_all[:, ts:ts + 1], y, Sb_sb[:, s:s + 1], None, ALU.mult)
            nc.vector.tensor_scalar(
                biasS_all[:, ts:ts + 1], y, bias_sb[:, kt, s:s + 1], None,
                ALU.mult)
            # xn = x - mean (bf16, dense) — ready for the final multiply
            xn = xnp.tile([128, H * W], BF16, tag="xn")
            nc.vector.tensor_scalar(
                xn[:].rearrange("p (r w) -> p r w", w=64),
                xr[:, 1:65, 2:66], mu_all[:, ts:ts + 1], None, ALU.subtract)
            return xn

        def emit_stage1(ts):
            s = ts // 4
            x16 = x16s[ts]
            p2 = p2p.tile([96, DE], BF16, tag="p2")
            # row-aligned 7-row slots; 2 slots per 2-bank psum tile
            r0 = 0
            ti = 0
            while r0 < NR:
                psA = psumA.tile([128, 1024], F32, tag="psA")
                rows = []
                for sub in (0, 512):
                    nr = min(7, NR - r0 - sum(rows))
                    if nr <= 0:
                        break
                    rows.append(nr)
                for si, nr in enumerate(rows):
                    rr = r0 + (rows[0] if si else 0)
                    lo = rr * PW
                    cw = nr * PW
                    for kw in range(3):
                        nc.tensor.matmul(
                            psA[0:96, si * 512: si * 512 + cw],
                            W1_sb[:, s, kw, :],
                            x16[:, lo + kw: lo + kw + cw],
                            start=(kw == 0), stop=(kw == 2))
                nrt = sum(rows)
                # seam-dropping copy: [96, slot, row, 68 -> 64] -> dense
                if len(rows) == 2 and rows[0] == rows[1]:
                    src = psA[0:96, :].rearrange(
                        "p (u q) -> p u q", q=512)[:, :, :rows[0] * PW] \
                        .rearrange("p u (r w) -> p u r w", w=PW)[:, :, :, 1:65]
                    dst = p2[:, r0 * 64: (r0 + nrt) * 64]
                    cop = nc.scalar.copy if ts % 2 == 0 else nc.vector.tensor_copy
                    cop(dst, src)
                else:
                    off = 0
                    for si, nr in enumerate(rows):
                        src = psA[0:96, si * 512: si * 512 + nr * PW].rearrange(
                            "p (r w) -> p r w", w=PW)[:, :, 1:65]
                        dst = p2[:, (r0 + off) * 64: (r0 + off + nr) * 64]
                        cop = (nc.scalar.copy if ts % 2 == 0
                               else nc.vector.tensor_copy)
                        cop(dst, src)
                        off += nr
                r0 += nrt
                ti += 1
            # pre-apply kh row shifts (dense pitch 64)
            nc.gpsimd.dma_start(p2[16:32, 0:DE - 64], p2[32:48, 64:DE])
            nc.gpsimd.dma_start(p2[32:48, 0:DE - 128], p2[64:80, 128:DE])
            return p2

        def emit_stage2(ts, p2, xn):
            predn = prednp.tile([128, H * W], BF16, tag="pred")
            out_sb = outp.tile([128, H * W], BF16, tag="out")
            for rt in range(0, 64, 14):
                nrt = min(14, 64 - rt)
                psB = psumB.tile([128, 1024], F32, tag="psB")
                slots = [(0, rt, min(7, nrt))]
                if nrt > 7:
                    slots.append((512, rt + 7, nrt - 7))
                for sub, r0, nr in slots:
                    nc.tensor.matmul(
                        psB[:, sub:sub + nr * 64], w2_sb[:],
                        p2[0:48, r0 * 64: (r0 + nr) * 64],
                        start=True, stop=True)
                if nrt == 14:
                    src = psB[:, :].rearrange(
                        "p (u q) -> p u q", q=512)[:, :, :448]
                    dst = predn[:, rt * 64: (rt + 14) * 64].rearrange(
                        "p (u q) -> p u q", q=448)
                    nc.scalar.activation(
                        dst, src, AF.Lrelu, bias=biasS_all[:, ts:ts + 1],
                        scale=scaleS_all[:, ts:ts + 1], alpha=0.01)
                else:
                    for sub, r0, nr in slots:
                        nc.scalar.activation(
                            predn[:, r0 * 64: (r0 + nr) * 64],
                            psB[:, sub:sub + nr * 64], AF.Lrelu,
                            bias=biasS_all[:, ts:ts + 1],
                            scale=scaleS_all[:, ts:ts + 1], alpha=0.01)
            nc.vector.tensor_tensor(out_sb[:], xn[:], predn[:], ALU.mult)
            for half in range(2):
                lo, hi = half * 2048, (half + 1) * 2048
                nc.sync.dma_start(out_d[ts][:, lo:hi], out_sb[:, lo:hi])

        # ---- software-pipelined main loop: stage1(k+1) ahead of stage2(k) ----
        xn0 = emit_stats(0)
        p2_prev, xn_prev = emit_stage1(0), xn0
        for k in range(8):
            if k + 1 < 8:
                if k + 3 < 8:
                    emit_xin(k + 3)
                xn_next = emit_stats(k + 1)
                p2_next = emit_stage1(k + 1)
            emit_stage2(k, p2_prev, xn_prev)
            if k + 1 < 8:
                p2_prev, xn_prev = p2_next, xn_next

    nc.compile()
    return nc


def _host_prep(style_encoding, content_in, dw_w, dw_b, pk_w, pk_b, pb_w, pb_b):
    """Shard + lay out inputs for the 8 cores (layout only, no math)."""
    f32 = np.float32
    bf = ml_dtypes.bfloat16
    common = {
        "dwT": np.ascontiguousarray(
            dw_w.reshape(8, 4, 128, 2, 2).transpose(2, 3, 4, 1, 0), f32),
        "dwb": np.ascontiguousarray(dw_b.reshape(8, 1), f32),
        "pbT": np.ascontiguousarray(
            pb_w.T.reshape(4, 128, 512).transpose(1, 0, 2), f32),
        "pbb": np.ascontiguousarray(pb_b.reshape(4, 128).T, f32),
        "pkwT": np.ascontiguousarray(
            pk_w.T.reshape(4, 128, 8).transpose(1, 0, 2), f32),
        "pkb": np.ascontiguousarray(pk_b.reshape(1, 8), f32),
    }
    ii = np.arange(128)
    common["mask32"] = (np.arange(32)[None, :] == (ii[:, None] // 8)).astype(bf)
    w2 = np.zeros((48, 128), bf)
    for kh in range(3):
        w2[kh * 16 + ii // 8, ii] = 1
    common["w2"] = w2
    common["repl8"] = (np.arange(8)[:, None] == (ii[None, :] % 8)).astype(f32)

    # padded pitch-68 bf16 content, all cores at once
    xp = np.pad(content_in, ((0, 0), (0, 0), (1, 1), (1, 1)), mode="reflect")
    buf = np.zeros((16, CH, NR, PW), f32)
    buf[:, :, :, 1:67] = xp
    xb = buf.reshape(16, 4, 128, EXT).astype(bf)

    in_maps = []
    for i in range(N_CORES):
        x16 = np.zeros((NSAMP, 4, 128, XAL), bf)
        x16[:, :, :, :EXT] = xb[NSAMP * i: NSAMP * (i + 1)]
        se = style_encoding[NSAMP * i: NSAMP * (i + 1)]
        in_maps.append({
            "x16": np.ascontiguousarray(x16.reshape(8, 128, XAL)),
            "style": np.ascontiguousarray(
                se.reshape(NSAMP, 4, 128, 16).transpose(2, 0, 1, 3), f32),
            **common,
        })
    return in_maps


def kernel(style_encoding, content_in, dw_w, dw_b, pk_w, pk_b, pb_w, pb_b):
    global LAST_RESULTS
    import os
    if "nc" not in _CACHE:
        _CACHE["nc"] = _build()
    nc = _CACHE["nc"]
    in_maps = _host_prep(style_encoding, content_in, dw_w, dw_b,
                         pk_w, pk_b, pb_w, pb_b)
    res = run_bass_kernel_spmd(
        nc, in_maps, core_ids=list(range(N_CORES)),
        trace=bool(os.environ.get("ADACONV_TRACE")))
    LAST_RESULTS = res
    outs = []
    for i in range(N_CORES):
        o = np.asarray(res.results[i]["out"]).astype(np.float32)
        outs.append(o.reshape(NSAMP, 4, 128, 64, 64).reshape(NSAMP, CH, 64, 64))
    return np.concatenate(outs, axis=0)
